# revision 1
# baseline (speedup 1.0000x reference)
"""Trainium2 Bass kernel for the Antecedent (fuzzy firing strength) problem.

fir[s, r] = exp(sum_d logmv[s, fs_ind[r, d], d])
with logmv[s, f, d] = -(x[s,d] - c[f,d])^2 / (2 * spread[f,d]^2)

The gather+sum over d is a matmul with contraction K = num_fs*in_dim = 32:
    fir[s, r] = exp( sum_k oh[k, r] * d2sq[k, s] ),
    oh[f*8+d, r]   = -1 iff fs_ind[r, d] == f, else 0 (host-built bf16 index
                     encoding; the -1 carries the gaussian exponent's sign)
    d2sq[f*8+d, s] = ((x[s,d]-c[f,d]) / (spread[f,d]*sqrt(2)))^2
                     (device-computed from x/center/spread)

Sharding: rules split across the 8 cores (8192 rules each); samples replicated.
Per core: 64 bf16 matmuls [K=32, M=128 samples, N=512 rules] -> f32 PSUM,
ScalarE Exp PSUM[128,2048] -> bf16 SBUF, 0.5MB DMAs to the [512, 8192] output
slice (bf16, upcast to f32 on the host). Steady state is ScalarE-bound: exp
runs at 1 elem/cycle/lane, 4.2M output elems/core ~= 31us, with matmul (~90%)
and output DMA (~60%) hidden under it.
"""

import sys

if "/opt/trn_rl_repo" not in sys.path:
    sys.path.insert(0, "/opt/trn_rl_repo")

import ml_dtypes
import numpy as np

import concourse.bacc as bacc
import concourse.mybir as mybir
import concourse.tile as tile
from concourse.bass_utils import run_bass_kernel_spmd
from concourse.tile_rust import add_dep_helper

NUM_SAM = 512
IN_DIM = 8
NUM_FS = 4
NUM_RULE = 65536
K = NUM_FS * IN_DIM  # 32 contraction size
N_CORES = 8
RPC = NUM_RULE // N_CORES  # 8192 rules per core

F32 = mybir.dt.float32
BF16 = mybir.dt.bfloat16
OUT_DT = BF16  # fir values are exp(<=0) in (0,1]; bf16 keeps rel err ~1e-3

# loop tiling (per core)
N_SG = NUM_SAM // 128          # 4 sample groups of 128 (partition dim)
N_MM = 4                       # matmuls per exp group (512 rules)
MM_N = 512                     # moving free dim per matmul
EXP_N = N_MM * MM_N            # 2048 rules per exp + output DMA group
N_GRP = RPC // EXP_N           # 4 groups per sample group


def build_nc(fact):
    nc = bacc.Bacc("TRN2", target_bir_lowering=False, debug=False, num_devices=N_CORES)

    oh_ext = nc.dram_tensor("onehot", [K, RPC], BF16, kind="ExternalInput")
    # xcs: cols 0..NUM_SAM-1 = x[s,d] repeated over f; col NUM_SAM = center,
    # col NUM_SAM+1 = spread (single input DMA for the whole prologue)
    xcs_ext = nc.dram_tensor("xcs", [K, NUM_SAM + 2], F32, kind="ExternalInput")
    if fact:
        oha_ext = nc.dram_tensor("oha", [K // 2, HI_PC], BF16, kind="ExternalInput")
        ohb_ext = nc.dram_tensor("ohb", [K // 2, N_LO], BF16, kind="ExternalInput")
    out_ext = nc.dram_tensor("out", [NUM_SAM, RPC], OUT_DT, kind="ExternalOutput")

    with tile.TileContext(nc) as tc:
        with (
            tc.tile_pool(name="const", bufs=1) as cpool,
            tc.tile_pool(name="stage", bufs=4) as spool,
            tc.tile_pool(name="psum", bufs=2, space="PSUM") as ppool,
        ):
            # ---- prologue: tiny inputs + membership table ----
            xcs = cpool.tile([K, NUM_SAM + 2], F32)
            nc.sync.dma_start(out=xcs[:], in_=xcs_ext[:])
            xt32 = xcs[:, 0:NUM_SAM]
            cvec = xcs[:, NUM_SAM : NUM_SAM + 1]
            svec = xcs[:, NUM_SAM + 1 : NUM_SAM + 2]

            if fact:
                # factor one-hots, plus a base-partition-0 copy of the
                # factor-B rows of xcs (matmul operands need bp 0)
                oha = cpool.tile([K // 2, HI_PC], BF16)
                ohb = cpool.tile([K // 2, N_LO], BF16)
                xcs2 = cpool.tile([K // 2, NUM_SAM + 2], F32)
                nc.sync.dma_start(out=oha[:], in_=oha_ext[:])
                nc.sync.dma_start(out=ohb[:], in_=ohb_ext[:])
                nc.sync.dma_start(out=xcs2[:], in_=xcs_ext[K // 2 : K, :])

            # one-hot rule encoding [K, RPC] with entries -1 (carries the
            # minus sign of the gaussian exponent); issued on the Scalar
            # HWDGE queue so it doesn't serialize behind Sync's const DMA
            oh = cpool.tile([K, RPC], BF16)
            if fact:
                # only the ACT-path groups (g=0,2 -> cols 0:2048, 4096:6144)
                # read oh; skip the DVE-path halves entirely
                chunks = [(0, MM_N), (MM_N, EXP_N - MM_N), (2 * EXP_N, EXP_N)]
            else:
                chunks = [(0, MM_N), (MM_N, 2560), (2688, 2560), (5248, 2944)]
            for c0, csz in chunks:  # small first chunk: first matmul sooner
                nc.scalar.dma_start(
                    out=oh[:, c0 : c0 + csz],
                    in_=oh_ext[:, c0 : c0 + csz],
                )

            # d2[k, s] = (x - c) / (s * sqrt(2)); lhsT = d2^2 (bf16).
            # The exponent's minus sign lives in the -1 one-hot entries.
            rsv = cpool.tile([K, 1], F32)
            tvec = cpool.tile([K, 1], F32)
            nc.vector.reciprocal(rsv[:], svec)
            nc.vector.tensor_scalar_mul(tvec[:], rsv[:], 0.7071067811865476)
            d2 = cpool.tile([K, NUM_SAM], F32)
            lhs_b = cpool.tile([K, NUM_SAM], BF16)
            # sample-group 0 first so its matmuls can start early
            for sl in (slice(0, 128), slice(128, NUM_SAM)):
                nc.vector.tensor_scalar(
                    d2[:, sl], xt32[:, sl], cvec, tvec[:],
                    mybir.AluOpType.subtract, mybir.AluOpType.mult,
                )
                nc.vector.tensor_mul(lhs_b[:, sl], d2[:, sl], d2[:, sl])

            Exp = mybir.ActivationFunctionType.Exp

            lhsB = None
            if fact:
                rsv2 = cpool.tile([K // 2, 1], F32)
                tvec2 = cpool.tile([K // 2, 1], F32)
                nc.vector.reciprocal(rsv2[:], xcs2[:, NUM_SAM + 1 : NUM_SAM + 2])
                nc.vector.tensor_scalar_mul(tvec2[:], rsv2[:], 0.7071067811865476)
                d2b = cpool.tile([K // 2, NUM_SAM], F32)
                lhsB = cpool.tile([K // 2, NUM_SAM], BF16)
                nc.vector.tensor_scalar(
                    d2b[:], xcs2[:, 0:NUM_SAM],
                    xcs2[:, NUM_SAM : NUM_SAM + 1], tvec2[:],
                    mybir.AluOpType.subtract, mybir.AluOpType.mult,
                )
                nc.vector.tensor_mul(lhsB[:], d2b[:], d2b[:])

            ab_tiles = []
            last_ab_mm = last_ab_exp = None
            if fact:
                # A/B tables: per sg, A sums at psum cols [sg*512, +32),
                # B sums at [sg*512+32, +288) (one bank per sg), then one
                # exp per sg -> ab_sb[sg] = [A (32 cols) | B (256 cols)] bf16
                ps_ab = ppool.tile([128, EXP_N], F32, tag="ps")
                for sg in range(N_SG):
                    s0 = sg * 512
                    nc.tensor.matmul(
                        ps_ab[:, s0 : s0 + HI_PC],
                        lhs_b[0 : K // 2, sg * 128 : (sg + 1) * 128],
                        oha[:],
                        start=True,
                        stop=True,
                    )
                    last_ab_mm = nc.tensor.matmul(
                        ps_ab[:, s0 + HI_PC : s0 + HI_PC + N_LO],
                        lhsB[:, sg * 128 : (sg + 1) * 128],
                        ohb[:],
                        start=True,
                        stop=True,
                    )
                for sg in range(N_SG):
                    ab = cpool.tile([128, HI_PC + N_LO], BF16, tag=f"ab{sg}")
                    s0 = sg * 512
                    last_ab_exp = nc.scalar.activation(
                        ab[:], ps_ab[:, s0 : s0 + HI_PC + N_LO], Exp
                    )
                    ab_tiles.append(ab)

            # ---- main loop ----
            # fact: groups 0,2 via onehot-matmul + ACT exp; groups 1,3 via
            # DVE broadcast multiply A[s,hi]*B[s,lo] (no exp, no big matmul)
            dve_groups = {1, 3} if fact else set()
            for sg in range(N_SG):
                lhsT = lhs_b[0:K, sg * 128 : (sg + 1) * 128]  # [32, 128]
                for g in range(N_GRP):
                    stg = spool.tile([128, EXP_N], OUT_DT)
                    out_slice = out_ext[
                        sg * 128 : (sg + 1) * 128, g * EXP_N : (g + 1) * EXP_N
                    ]
                    if g in dve_groups:
                        ab = ab_tiles[sg]
                        Ab = (
                            ab[:, g * 8 : (g + 1) * 8]
                            .rearrange("p (h o) -> p h o", o=1)
                            .broadcast_to([128, 8, N_LO])
                        )
                        Bb = (
                            ab[:, HI_PC : HI_PC + N_LO]
                            .rearrange("p (o n) -> p o n", o=1)
                            .broadcast_to([128, 8, N_LO])
                        )
                        o3 = stg[:].rearrange("p (h n) -> p h n", h=8)
                        nc.vector.tensor_tensor(o3, Bb, Ab, mybir.AluOpType.mult)
                        nc.sync.dma_start(out=out_slice, in_=stg[:])
                        continue
                    ps = ppool.tile([128, EXP_N], F32, tag="ps")
                    for j in range(N_MM):
                        rt = g * N_MM + j
                        mm = nc.tensor.matmul(
                            ps[:, j * MM_N : (j + 1) * MM_N],
                            lhsT,
                            oh[:, rt * MM_N : (rt + 1) * MM_N],
                            start=True,
                            stop=True,
                        )
                        if last_ab_mm is not None:
                            # keep the AB phase ahead of the loop on the PE
                            # stream so the shared psum slot can't deadlock
                            add_dep_helper(
                                mm.ins, last_ab_mm.ins, sync=False,
                                reason="AB tables before rule matmuls",
                            )
                            last_ab_mm = None
                    if last_ab_exp is not None:
                        ex = nc.scalar.activation(
                            stg[:, 0:MM_N], ps[:, 0:MM_N], Exp
                        )
                        add_dep_helper(
                            ex.ins, last_ab_exp.ins, sync=False,
                            reason="AB exps before rule exps",
                        )
                        last_ab_exp = None
                        nc.scalar.activation(stg[:, MM_N:], ps[:, MM_N:], Exp)
                        nc.sync.dma_start(out=out_slice, in_=stg[:])
                    elif sg == N_SG - 1 and g == N_GRP - 1:
                        nc.scalar.activation(stg[:], ps[:], Exp)
                        # two half DMAs run concurrently -> shorter drain tail
                        h = EXP_N // 2
                        nc.sync.dma_start(out=out_slice[:, :h], in_=stg[:, :h])
                        nc.sync.dma_start(out=out_slice[:, h:], in_=stg[:, h:])
                    else:
                        nc.scalar.activation(stg[:], ps[:], Exp)
                        nc.sync.dma_start(out=out_slice, in_=stg[:])

    nc.compile()
    return nc


KX = K + K // 2       # xcs rows: 32 standard + 16 duplicated factor-B rows
D_A = IN_DIM // 2     # factor A: dims 0..3 (k rows 0..15)
N_HI = NUM_FS**D_A    # 256 A-codes; per core 32 hi blocks
N_LO = NUM_FS**D_A    # 256 B-codes
HI_PC = RPC // N_LO   # 32 hi blocks per core


def _is_factorizable(fs):
    """fs[r, 0:4] depends only on r>>8 and fs[r, 4:8] only on r&255
    (true for the FuCo-FRB cartesian rule base)."""
    a = fs[:, :D_A].reshape(N_HI, N_LO, D_A)
    b = fs[:, D_A:].reshape(N_HI, N_LO, D_A)
    return bool((a == a[:, :1]).all() and (b == b[:1]).all())


def _prep_in_maps(model_input, center, spread, fs_ind):
    model_input = np.ascontiguousarray(model_input, dtype=np.float32)
    center = np.ascontiguousarray(center, dtype=np.float32)
    spread = np.ascontiguousarray(spread, dtype=np.float32)
    fs = np.clip(np.asarray(fs_ind), 0, NUM_FS - 1).astype(np.int64)

    # one-hot with k = d*NUM_FS + f rows: oh[k, r] = -1 iff fs_ind[r, d] == f
    # (the -1 carries the gaussian exponent's sign)
    oh = np.zeros((K, NUM_RULE), dtype=ml_dtypes.bfloat16)
    r = np.arange(NUM_RULE)
    for d in range(IN_DIM):
        oh[d * NUM_FS + fs[:, d], r] = -1.0

    # xcs: x transposed/repeated over f, plus center and spread columns
    # (row k = d*4+f holds x[s, d], center[f, d], spread[f, d]).
    # Rows 32..47 duplicate rows 16..31 (factor-B dims) so the fast path's
    # K=16 B-matmul can run at base partition 32 (tile_position constraint).
    xcs = np.empty((K, NUM_SAM + 2), dtype=np.float32)
    xcs[:, :NUM_SAM] = np.repeat(model_input.T, NUM_FS, axis=0)
    xcs[:, NUM_SAM] = center.T.reshape(K)
    xcs[:, NUM_SAM + 1] = spread.T.reshape(K)

    fact = _is_factorizable(fs)
    oha = ohb = None
    if fact:
        # A-table one-hot [16, 256 hi codes], B-table one-hot [16, 256]
        oha = np.zeros((K // 2, N_HI), dtype=ml_dtypes.bfloat16)
        ohb = np.zeros((K // 2, N_LO), dtype=ml_dtypes.bfloat16)
        hi_rep = fs[:: N_LO, :D_A]  # [256, 4] representative rows
        lo_rep = fs[:N_LO, D_A:]    # [256, 4]
        for d in range(D_A):
            oha[d * NUM_FS + hi_rep[:, d], np.arange(N_HI)] = -1.0
            ohb[d * NUM_FS + lo_rep[:, d], np.arange(N_LO)] = -1.0

    maps = []
    for i in range(N_CORES):
        m = {
            "onehot": np.ascontiguousarray(oh[:, i * RPC : (i + 1) * RPC]),
            "xcs": xcs,
        }
        if fact:
            m["oha"] = np.ascontiguousarray(oha[:, i * HI_PC : (i + 1) * HI_PC])
            m["ohb"] = ohb
        maps.append(m)
    return fact, maps


def _run(inputs, trace=False, **spmd_kwargs):
    fact, in_maps = _prep_in_maps(
        inputs["model_input"], inputs["center"], inputs["spread"], inputs["fs_ind"]
    )
    nc = build_nc(fact)
    res = run_bass_kernel_spmd(
        nc, in_maps, core_ids=list(range(N_CORES)), trace=trace, **spmd_kwargs
    )
    out = np.concatenate(
        [res.results[i]["out"].astype(np.float32) for i in range(N_CORES)], axis=1
    )
    return out, res


def kernel(model_input, center, spread, fs_ind):
    out, _ = _run(
        {
            "model_input": model_input,
            "center": center,
            "spread": spread,
            "fs_ind": fs_ind,
        }
    )
    return out



# revision 4
# speedup vs baseline: 1.0658x; 1.0658x over previous
"""Trainium2 Bass kernel for the Antecedent (fuzzy firing strength) problem.

fir[s, r] = exp(sum_d logmv[s, fs_ind[r, d], d])
with logmv[s, f, d] = -(x[s,d] - c[f,d])^2 / (2 * spread[f,d]^2)

For the FuCo-FRB cartesian rule base, fs_ind factorizes: fs_ind[r, 0:4]
depends only on hi = r>>8 and fs_ind[r, 4:8] only on lo = r&255, so
    fir[s, r] = A[s, hi] * B[s, lo]
with A = exp(onehotA @ d2sqA), B = exp(onehotB @ d2sqB) tiny per-sample
tables (per core: 32 hi codes, 256 lo codes).

Sharding: rules split across the 8 cores (8192 rules each); samples
replicated. The kernel is output-DMA bound (8 MB bf16 per core ~ 24 us
at the ~358 GB/s per-core HBM limit), so the structure keeps the store
DMA saturated from ~10 us on:
  - 10 of 16 [128, 2048] output groups are produced on the Vector engine
    as 8x tensor_scalar_mul(B_block[128,256], scalar=A[:,h]) in bf16
    (4x DVE mode, ~0.13 us/op), DMA'd via the Sync HWDGE queue;
  - 6 groups go through the one-hot matmul (K=32) + ScalarE Exp path,
    DMA'd via the Scalar HWDGE queue right after their exp in-stream;
  - first/last groups use 512-col split DMAs to pull the DMA start
    earlier and shrink the drain tail.
Output is bf16 (fir in (0,1]; rel err ~2e-3), upcast to f32 on the host.
"""

import sys

if "/opt/trn_rl_repo" not in sys.path:
    sys.path.insert(0, "/opt/trn_rl_repo")

import ml_dtypes
import numpy as np

import concourse.bacc as bacc
import concourse.mybir as mybir
import concourse.tile as tile
from concourse.bass_utils import run_bass_kernel_spmd

NUM_SAM = 512
IN_DIM = 8
NUM_FS = 4
NUM_RULE = 65536
K = NUM_FS * IN_DIM  # 32 contraction size
N_CORES = 8
RPC = NUM_RULE // N_CORES  # 8192 rules per core

F32 = mybir.dt.float32
BF16 = mybir.dt.bfloat16
OUT_DT = BF16

N_SG = NUM_SAM // 128  # 4 sample groups of 128 (partition dim)
N_MM = 4               # matmuls per exp group (512 rules each)
MM_N = 512
EXP_N = N_MM * MM_N    # 2048 rules per group
N_GRP = RPC // EXP_N   # 4 rule groups per sample group

D_A = IN_DIM // 2      # factor A: dims 0..3 (k rows 0..15)
N_HI = NUM_FS**D_A     # 256 A-codes globally
N_LO = NUM_FS**D_A     # 256 B-codes
HI_PC = RPC // N_LO    # 32 hi codes per core

XCS_W = NUM_SAM + 2    # x cols + center col + spread col

# (sg, g) slots handled by the matmul+exp path; the rest go through the
# DVE tensor_scalar path. ACT leans on later sgs so DVE isn't the tail.
ACT_SLOTS = ((0, 3), (1, 3), (2, 2), (2, 3), (3, 2), (3, 3))
ACT_GS = (3, 2)        # distinct rule-groups the one-hot input covers, g3 first

RSQRT2 = 0.7071067811865476
Exp = mybir.ActivationFunctionType.Exp


def _prep_lhs(nc, cpool, src_tile, nrow):
    """d2[k, s] = ((x - c) / (s*sqrt(2)))^2 as bf16 [nrow, NUM_SAM]."""
    rsv = cpool.tile([nrow, 1], F32, name=f"rsv{nrow}")
    tvec = cpool.tile([nrow, 1], F32, name=f"tvec{nrow}")
    nc.vector.reciprocal(rsv[:], src_tile[:, NUM_SAM + 1 : NUM_SAM + 2])
    nc.vector.tensor_scalar_mul(tvec[:], rsv[:], RSQRT2)
    d2 = cpool.tile([nrow, NUM_SAM], F32, name=f"d2{nrow}")
    lhs = cpool.tile([nrow, NUM_SAM], BF16, name=f"lhs{nrow}")
    nc.vector.tensor_scalar(
        d2[:], src_tile[:, 0:NUM_SAM],
        src_tile[:, NUM_SAM : NUM_SAM + 1], tvec[:],
        mybir.AluOpType.subtract, mybir.AluOpType.mult,
    )
    nc.vector.tensor_mul(lhs[:], d2[:], d2[:])
    return lhs


def build_fact():
    nc = bacc.Bacc("TRN2", target_bir_lowering=False, debug=False, num_devices=N_CORES)

    xcs_ext = nc.dram_tensor("xcs", [K, XCS_W], F32, kind="ExternalInput")
    xcsb_ext = nc.dram_tensor("xcsb", [K // 2, XCS_W], F32, kind="ExternalInput")
    ohab_ext = nc.dram_tensor("ohab", [K // 2, HI_PC + N_LO], BF16, kind="ExternalInput")
    # one-hot for the ACT-path rule groups only, packed [g=3 | g=2]
    ohact_ext = nc.dram_tensor("ohact", [K, len(ACT_GS) * EXP_N], BF16, kind="ExternalInput")
    out_ext = nc.dram_tensor("out", [NUM_SAM, RPC], OUT_DT, kind="ExternalOutput")

    with tile.TileContext(nc) as tc:
        with (
            tc.tile_pool(name="const", bufs=1) as cpool,
            tc.tile_pool(name="stgv", bufs=3) as svp,
            tc.tile_pool(name="stga", bufs=2) as sap,
            tc.tile_pool(name="psum", bufs=2, space="PSUM") as ppool,
        ):
            # ---- input DMAs, spread across issue queues ----
            xcs = cpool.tile([K, XCS_W], F32)
            nc.scalar.dma_start(out=xcs[:], in_=xcs_ext[:])
            xcsb = cpool.tile([K // 2, XCS_W], F32)
            nc.gpsimd.dma_start(out=xcsb[:], in_=xcsb_ext[:])
            ohab = cpool.tile([K // 2, HI_PC + N_LO], BF16)
            nc.sync.dma_start(out=ohab[:], in_=ohab_ext[:])
            ohact = cpool.tile([K, len(ACT_GS) * EXP_N], BF16)
            for ci in range(len(ACT_GS)):  # g3 chunk first (used first)
                nc.sync.dma_start(
                    out=ohact[:, ci * EXP_N : (ci + 1) * EXP_N],
                    in_=ohact_ext[:, ci * EXP_N : (ci + 1) * EXP_N],
                )

            # ---- squared scaled distances (matmul lhs), bf16 ----
            lhs = _prep_lhs(nc, cpool, xcs, K)        # rows 0..31, bp0
            lhsb = _prep_lhs(nc, cpool, xcsb, K // 2)  # dup B rows at bp0

            # ---- A/B tables per sample group ----
            # per sg the AB sums live in their own psum bank: cols sg*512..
            ps_ab = ppool.tile([128, EXP_N], F32, tag="ps")
            for sg in range(N_SG):
                s0 = sg * MM_N
                sl = slice(sg * 128, (sg + 1) * 128)
                nc.tensor.matmul(
                    ps_ab[:, s0 : s0 + HI_PC],
                    lhs[0 : K // 2, sl], ohab[:, 0:HI_PC],
                    start=True, stop=True,
                )
                nc.tensor.matmul(
                    ps_ab[:, s0 + HI_PC : s0 + HI_PC + N_LO],
                    lhsb[:, sl], ohab[:, HI_PC : HI_PC + N_LO],
                    start=True, stop=True,
                )
            a_tiles, b_tiles = [], []
            for sg in range(N_SG):
                # A as f32 (tensor_scalar's scalar operand must be f32),
                # B as bf16 (keeps the 4x DVE mode on the streamed block)
                af = cpool.tile([128, HI_PC], F32, name=f"af{sg}")
                bb = cpool.tile([128, N_LO], BF16, name=f"bb{sg}")
                s0 = sg * MM_N
                nc.scalar.activation(af[:], ps_ab[:, s0 : s0 + HI_PC], Exp)
                nc.scalar.activation(
                    bb[:], ps_ab[:, s0 + HI_PC : s0 + HI_PC + N_LO], Exp
                )
                a_tiles.append(af)
                b_tiles.append(bb)

            # ---- main loop: produce 16 [128, 2048] groups ----
            act_slots = set(ACT_SLOTS)
            g_of_chunk = {g: ci for ci, g in enumerate(ACT_GS)}

            dve_order = [
                (sg, g)
                for sg in range(N_SG)
                for g in range(N_GRP)
                if (sg, g) not in act_slots
            ]
            act_order = list(ACT_SLOTS)

            def emit_dve(sg, g, split):
                af = a_tiles[sg]
                bblk = b_tiles[sg][:]
                stg = svp.tile([128, EXP_N], OUT_DT, name=f"sv{sg}_{g}")
                orow = out_ext[sg * 128 : (sg + 1) * 128, g * EXP_N : (g + 1) * EXP_N]
                for h in range(8):
                    c = g * 8 + h
                    nc.vector.tensor_scalar_mul(
                        stg[:, h * N_LO : (h + 1) * N_LO], bblk, af[:, c : c + 1]
                    )
                    if split and h % 2 == 1:
                        nc.sync.dma_start(
                            out=orow[:, (h - 1) * N_LO : (h + 1) * N_LO],
                            in_=stg[:, (h - 1) * N_LO : (h + 1) * N_LO],
                        )
                if not split:
                    nc.sync.dma_start(out=orow, in_=stg[:])

            def emit_act(sg, g, split):
                ci = g_of_chunk[g]
                lhsT = lhs[:, sg * 128 : (sg + 1) * 128]
                ps = ppool.tile([128, EXP_N], F32, tag="ps", name=f"ps{sg}_{g}")
                for j in range(N_MM):
                    nc.tensor.matmul(
                        ps[:, j * MM_N : (j + 1) * MM_N],
                        lhsT,
                        ohact[:, ci * EXP_N + j * MM_N : ci * EXP_N + (j + 1) * MM_N],
                        start=True, stop=True,
                    )
                stg = sap.tile([128, EXP_N], OUT_DT, name=f"sa{sg}_{g}")
                orow = out_ext[sg * 128 : (sg + 1) * 128, g * EXP_N : (g + 1) * EXP_N]
                if split:
                    h = EXP_N // 2
                    nc.scalar.activation(stg[:, 0:h], ps[:, 0:h], Exp)
                    nc.scalar.dma_start(out=orow[:, 0:h], in_=stg[:, 0:h])
                    nc.scalar.activation(stg[:, h:], ps[:, h:], Exp)
                    nc.scalar.dma_start(out=orow[:, h:], in_=stg[:, h:])
                else:
                    nc.scalar.activation(stg[:], ps[:], Exp)
                    nc.scalar.dma_start(out=orow, in_=stg[:])

            # interleave emission: 2 DVE groups per ACT group keeps both
            # engine streams flowing; first DVE and both tails use split DMA
            di = ai = 0
            while di < len(dve_order) or ai < len(act_order):
                for _ in range(2):
                    if di < len(dve_order):
                        sg, g = dve_order[di]
                        emit_dve(sg, g, split=(di == 0 or di == len(dve_order) - 1))
                        di += 1
                if ai < len(act_order):
                    sg, g = act_order[ai]
                    emit_act(sg, g, split=(ai == len(act_order) - 1))
                    ai += 1

    nc.compile()
    return nc


def build_nofact():
    """Fallback for a non-factorizable rule base: one-hot matmul + exp
    for all 16 groups (the previously validated path)."""
    nc = bacc.Bacc("TRN2", target_bir_lowering=False, debug=False, num_devices=N_CORES)

    oh_ext = nc.dram_tensor("onehot", [K, RPC], BF16, kind="ExternalInput")
    xcs_ext = nc.dram_tensor("xcs", [K, XCS_W], F32, kind="ExternalInput")
    out_ext = nc.dram_tensor("out", [NUM_SAM, RPC], OUT_DT, kind="ExternalOutput")

    with tile.TileContext(nc) as tc:
        with (
            tc.tile_pool(name="const", bufs=1) as cpool,
            tc.tile_pool(name="stage", bufs=4) as spool,
            tc.tile_pool(name="psum", bufs=2, space="PSUM") as ppool,
        ):
            xcs = cpool.tile([K, XCS_W], F32)
            nc.sync.dma_start(out=xcs[:], in_=xcs_ext[:])

            oh = cpool.tile([K, RPC], BF16)
            chunks = [(0, MM_N), (MM_N, 2560), (2688, 2560), (5248, 2944)]
            for c0, csz in chunks:
                nc.scalar.dma_start(
                    out=oh[:, c0 : c0 + csz], in_=oh_ext[:, c0 : c0 + csz]
                )

            lhs = _prep_lhs(nc, cpool, xcs, K)

            for sg in range(N_SG):
                lhsT = lhs[:, sg * 128 : (sg + 1) * 128]
                for g in range(N_GRP):
                    stg = spool.tile([128, EXP_N], OUT_DT)
                    out_slice = out_ext[
                        sg * 128 : (sg + 1) * 128, g * EXP_N : (g + 1) * EXP_N
                    ]
                    ps = ppool.tile([128, EXP_N], F32, tag="ps")
                    for j in range(N_MM):
                        rt = g * N_MM + j
                        nc.tensor.matmul(
                            ps[:, j * MM_N : (j + 1) * MM_N],
                            lhsT,
                            oh[:, rt * MM_N : (rt + 1) * MM_N],
                            start=True, stop=True,
                        )
                    nc.scalar.activation(stg[:], ps[:], Exp)
                    if sg == N_SG - 1 and g == N_GRP - 1:
                        h = EXP_N // 2
                        nc.sync.dma_start(out=out_slice[:, :h], in_=stg[:, :h])
                        nc.sync.dma_start(out=out_slice[:, h:], in_=stg[:, h:])
                    else:
                        nc.sync.dma_start(out=out_slice, in_=stg[:])

    nc.compile()
    return nc


def _is_factorizable(fs):
    """fs[r, 0:4] depends only on r>>8 and fs[r, 4:8] only on r&255."""
    a = fs[:, :D_A].reshape(N_HI, N_LO, D_A)
    b = fs[:, D_A:].reshape(N_HI, N_LO, D_A)
    return bool((a == a[:, :1]).all() and (b == b[:1]).all())


def _prep_in_maps(model_input, center, spread, fs_ind):
    model_input = np.ascontiguousarray(model_input, dtype=np.float32)
    center = np.ascontiguousarray(center, dtype=np.float32)
    spread = np.ascontiguousarray(spread, dtype=np.float32)
    fs = np.clip(np.asarray(fs_ind), 0, NUM_FS - 1).astype(np.int64)

    # xcs row k = d*4+f holds x[s, d] (cols 0:512), center[f, d], spread[f, d]
    xcs = np.empty((K, XCS_W), dtype=np.float32)
    xcs[:, :NUM_SAM] = np.repeat(model_input.T, NUM_FS, axis=0)
    xcs[:, NUM_SAM] = center.T.reshape(K)
    xcs[:, NUM_SAM + 1] = spread.T.reshape(K)

    fact = _is_factorizable(fs)
    r = np.arange(NUM_RULE)

    maps = []
    if fact:
        xcsb = np.ascontiguousarray(xcs[K // 2 : K, :])
        # factor one-hots: entries -1 (carry the gaussian exponent's sign)
        oha = np.zeros((K // 2, N_HI), dtype=ml_dtypes.bfloat16)
        ohb = np.zeros((K // 2, N_LO), dtype=ml_dtypes.bfloat16)
        hi_rep = fs[::N_LO, :D_A]
        lo_rep = fs[:N_LO, D_A:]
        for d in range(D_A):
            oha[d * NUM_FS + hi_rep[:, d], np.arange(N_HI)] = -1.0
            ohb[d * NUM_FS + lo_rep[:, d], np.arange(N_LO)] = -1.0
        # full one-hot only for the ACT-path rule-group columns
        oh = np.zeros((K, NUM_RULE), dtype=ml_dtypes.bfloat16)
        for d in range(IN_DIM):
            oh[d * NUM_FS + fs[:, d], r] = -1.0
        for i in range(N_CORES):
            ohab = np.concatenate(
                [oha[:, i * HI_PC : (i + 1) * HI_PC], ohb], axis=1
            )
            ohact = np.concatenate(
                [
                    oh[:, i * RPC + g * EXP_N : i * RPC + (g + 1) * EXP_N]
                    for g in ACT_GS
                ],
                axis=1,
            )
            maps.append(
                {
                    "xcs": xcs,
                    "xcsb": xcsb,
                    "ohab": np.ascontiguousarray(ohab),
                    "ohact": np.ascontiguousarray(ohact),
                }
            )
    else:
        oh = np.zeros((K, NUM_RULE), dtype=ml_dtypes.bfloat16)
        for d in range(IN_DIM):
            oh[d * NUM_FS + fs[:, d], r] = -1.0
        for i in range(N_CORES):
            maps.append(
                {
                    "onehot": np.ascontiguousarray(oh[:, i * RPC : (i + 1) * RPC]),
                    "xcs": xcs,
                }
            )
    return fact, maps


def _run(inputs, trace=False, **spmd_kwargs):
    fact, in_maps = _prep_in_maps(
        inputs["model_input"], inputs["center"], inputs["spread"], inputs["fs_ind"]
    )
    nc = build_fact() if fact else build_nofact()
    res = run_bass_kernel_spmd(
        nc, in_maps, core_ids=list(range(N_CORES)), trace=trace, **spmd_kwargs
    )
    out = np.concatenate(
        [res.results[i]["out"].astype(np.float32) for i in range(N_CORES)], axis=1
    )
    return out, res


def kernel(model_input, center, spread, fs_ind):
    out, _ = _run(
        {
            "model_input": model_input,
            "center": center,
            "spread": spread,
            "fs_ind": fs_ind,
        }
    )
    return out


# revision 6
# speedup vs baseline: 1.0691x; 1.0031x over previous
"""Trainium2 Bass kernel for the Antecedent (fuzzy firing strength) problem.

fir[s, r] = exp(sum_d logmv[s, fs_ind[r, d], d])
with logmv[s, f, d] = -(x[s,d] - c[f,d])^2 / (2 * spread[f,d]^2)

For the FuCo-FRB cartesian rule base, fs_ind factorizes: fs_ind[r, 0:4]
depends only on hi = r>>8 and fs_ind[r, 4:8] only on lo = r&255, so
    fir[s, r] = A[s, hi] * B[s, lo]
with A, B tiny per-sample tables (per core: 32 hi codes, 256 lo codes)
computed via one-hot matmuls + exp.

Sharding: rules split across the 8 cores (8192 rules each); samples
replicated. Production of the 16 [128, 2048] output groups per core is
split across engines so both stores and compute stream continuously:
  - groups g=0,1 (8): VectorE broadcast multiply A[s,hi]*B[s,lo]
    (TENSOR_TENSOR, one op per group), stored via the Sync HWDGE queue;
  - groups g=2,3 (8): TensorE one-hot matmul (K=32) + ScalarE Exp,
    stored via the GpSimd SWDGE queue (ScalarE does pure compute;
    the idle Pool engine is the DMA clerk);
  - 14 warmup matmuls prime the PE out of its cold HAM clock before
    the real matmul stream begins;
  - first/last groups use split ops + split DMAs to pull the first
    store earlier and shrink the drain tail.
Output is bf16 (fir in (0,1]; rel err ~2e-3), upcast to f32 on the host.
"""

import sys

if "/opt/trn_rl_repo" not in sys.path:
    sys.path.insert(0, "/opt/trn_rl_repo")

import ml_dtypes
import numpy as np

import concourse.bacc as bacc
import concourse.mybir as mybir
import concourse.tile as tile
from concourse.bass_utils import run_bass_kernel_spmd

NUM_SAM = 512
IN_DIM = 8
NUM_FS = 4
NUM_RULE = 65536
K = NUM_FS * IN_DIM  # 32 contraction size
N_CORES = 8
RPC = NUM_RULE // N_CORES  # 8192 rules per core

F32 = mybir.dt.float32
BF16 = mybir.dt.bfloat16
OUT_DT = BF16

N_SG = NUM_SAM // 128  # 4 sample groups of 128 (partition dim)
N_MM = 4               # matmuls per exp group (512 rules each)
MM_N = 512
EXP_N = N_MM * MM_N    # 2048 rules per group
N_GRP = RPC // EXP_N   # 4 rule groups per sample group

D_A = IN_DIM // 2      # factor A: dims 0..3 (k rows 0..15)
N_HI = NUM_FS**D_A     # 256 A-codes globally
N_LO = NUM_FS**D_A     # 256 B-codes
HI_PC = RPC // N_LO    # 32 hi codes per core

XCS_W = NUM_SAM + 2    # x cols + center col + spread col

ACT_GS = (2, 3)        # rule-groups on the matmul+exp path (all sgs)
N_WARM = 14            # PE warmup matmuls

RSQRT2 = 0.7071067811865476
Exp = mybir.ActivationFunctionType.Exp
Mult = mybir.AluOpType.mult


def _prep_lhs(nc, cpool, src_tile, nrow):
    """d2[k, s] = ((x - c) / (s*sqrt(2)))^2 as bf16 [nrow, NUM_SAM]."""
    rsv = cpool.tile([nrow, 1], F32, name=f"rsv{nrow}")
    tvec = cpool.tile([nrow, 1], F32, name=f"tvec{nrow}")
    nc.vector.reciprocal(rsv[:], src_tile[:, NUM_SAM + 1 : NUM_SAM + 2])
    nc.vector.tensor_scalar_mul(tvec[:], rsv[:], RSQRT2)
    d2 = cpool.tile([nrow, NUM_SAM], F32, name=f"d2{nrow}")
    lhs = cpool.tile([nrow, NUM_SAM], BF16, name=f"lhs{nrow}")
    nc.vector.tensor_scalar(
        d2[:], src_tile[:, 0:NUM_SAM],
        src_tile[:, NUM_SAM : NUM_SAM + 1], tvec[:],
        mybir.AluOpType.subtract, Mult,
    )
    nc.vector.tensor_mul(lhs[:], d2[:], d2[:])
    return lhs


def build_fact():
    nc = bacc.Bacc("TRN2", target_bir_lowering=False, debug=False, num_devices=N_CORES)

    xcs_ext = nc.dram_tensor("xcs", [K, XCS_W], F32, kind="ExternalInput")
    xcsb_ext = nc.dram_tensor("xcsb", [K // 2, XCS_W], F32, kind="ExternalInput")
    ohab_ext = nc.dram_tensor("ohab", [K // 2, HI_PC + N_LO], BF16, kind="ExternalInput")
    # one-hot for the ACT-path rule groups, packed in ACT_GS order
    ohact_ext = nc.dram_tensor("ohact", [K, len(ACT_GS) * EXP_N], BF16, kind="ExternalInput")
    out_ext = nc.dram_tensor("out", [NUM_SAM, RPC], OUT_DT, kind="ExternalOutput")

    with tile.TileContext(nc) as tc:
        with (
            tc.tile_pool(name="const", bufs=1) as cpool,
            tc.tile_pool(name="stgv", bufs=3) as svp,
            tc.tile_pool(name="stga", bufs=3) as sap,
            tc.tile_pool(name="psum", bufs=2, space="PSUM") as ppool,
        ):
            # ---- input DMAs, spread across issue queues ----
            xcs = cpool.tile([K, XCS_W], F32)
            nc.scalar.dma_start(out=xcs[:], in_=xcs_ext[:])
            xcsb = cpool.tile([K // 2, XCS_W], F32)
            nc.gpsimd.dma_start(out=xcsb[:], in_=xcsb_ext[:])
            ohab = cpool.tile([K // 2, HI_PC + N_LO], BF16)
            nc.sync.dma_start(out=ohab[:], in_=ohab_ext[:])
            ohact = cpool.tile([K, len(ACT_GS) * EXP_N], BF16)
            for ci in range(len(ACT_GS)):
                nc.sync.dma_start(
                    out=ohact[:, ci * EXP_N : (ci + 1) * EXP_N],
                    in_=ohact_ext[:, ci * EXP_N : (ci + 1) * EXP_N],
                )

            # ---- PE warmup: prime the HAM activity window so the real
            # matmul stream runs at the 2.4 GHz warm clock ----
            wsrc = cpool.tile([K // 2, 128], BF16)
            nc.gpsimd.memset(wsrc[:], 0)
            ps_w = ppool.tile([128, EXP_N], F32, tag="ps")
            for i in range(N_WARM):
                j = i % (EXP_N // 128)
                nc.tensor.matmul(
                    ps_w[:, j * 128 : (j + 1) * 128],
                    wsrc[:], wsrc[:], start=True, stop=True,
                )

            # ---- squared scaled distances (matmul lhs), bf16 ----
            lhs = _prep_lhs(nc, cpool, xcs, K)         # rows 0..31, bp0
            lhsb = _prep_lhs(nc, cpool, xcsb, K // 2)  # dup B rows at bp0

            # ---- A/B tables per sample group ----
            ps_ab = ppool.tile([128, EXP_N], F32, tag="ps")
            for sg in range(N_SG):
                s0 = sg * MM_N
                sl = slice(sg * 128, (sg + 1) * 128)
                nc.tensor.matmul(
                    ps_ab[:, s0 : s0 + HI_PC],
                    lhs[0 : K // 2, sl], ohab[:, 0:HI_PC],
                    start=True, stop=True,
                )
                nc.tensor.matmul(
                    ps_ab[:, s0 + HI_PC : s0 + HI_PC + N_LO],
                    lhsb[:, sl], ohab[:, HI_PC : HI_PC + N_LO],
                    start=True, stop=True,
                )
            ab_tiles = []
            for sg in range(N_SG):
                ab = cpool.tile([128, HI_PC + N_LO], BF16, name=f"ab{sg}")
                s0 = sg * MM_N
                nc.scalar.activation(ab[:], ps_ab[:, s0 : s0 + HI_PC + N_LO], Exp)
                ab_tiles.append(ab)

            # ---- main loop ----
            def dve_tt(stg, ab, g, h0, nh):
                """stg[:, h0*256:(h0+nh)*256] = A[:, g*8+h] * B  via one TT."""
                Ab = (
                    ab[:, g * 8 + h0 : g * 8 + h0 + nh]
                    .rearrange("p (h o) -> p h o", o=1)
                    .broadcast_to([128, nh, N_LO])
                )
                Bb = (
                    ab[:, HI_PC : HI_PC + N_LO]
                    .rearrange("p (o n) -> p o n", o=1)
                    .broadcast_to([128, nh, N_LO])
                )
                o3 = stg[:, h0 * N_LO : (h0 + nh) * N_LO].rearrange(
                    "p (h n) -> p h n", h=nh
                )
                nc.vector.tensor_tensor(o3, Bb, Ab, Mult)

            def emit_dve(sg, g, nsplit):
                ab = ab_tiles[sg]
                stg = svp.tile([128, EXP_N], OUT_DT, name="svstg")
                orow = out_ext[sg * 128 : (sg + 1) * 128, g * EXP_N : (g + 1) * EXP_N]
                hs = 8 // nsplit
                for p in range(nsplit):
                    dve_tt(stg, ab, g, p * hs, hs)
                    nc.sync.dma_start(
                        out=orow[:, p * hs * N_LO : (p + 1) * hs * N_LO],
                        in_=stg[:, p * hs * N_LO : (p + 1) * hs * N_LO],
                    )

            def emit_act(sg, g, nsplit):
                ci = ACT_GS.index(g)
                lhsT = lhs[:, sg * 128 : (sg + 1) * 128]
                ps = ppool.tile([128, EXP_N], F32, tag="ps", name="ps")
                for j in range(N_MM):
                    nc.tensor.matmul(
                        ps[:, j * MM_N : (j + 1) * MM_N],
                        lhsT,
                        ohact[:, ci * EXP_N + j * MM_N : ci * EXP_N + (j + 1) * MM_N],
                        start=True, stop=True,
                    )
                stg = sap.tile([128, EXP_N], OUT_DT, name="sastg")
                orow = out_ext[sg * 128 : (sg + 1) * 128, g * EXP_N : (g + 1) * EXP_N]
                w = EXP_N // nsplit
                for p in range(nsplit):
                    nc.scalar.activation(
                        stg[:, p * w : (p + 1) * w], ps[:, p * w : (p + 1) * w], Exp
                    )
                    nc.gpsimd.dma_start(
                        out=orow[:, p * w : (p + 1) * w],
                        in_=stg[:, p * w : (p + 1) * w],
                    )

            for sg in range(N_SG):
                for g in range(N_GRP):
                    first = sg == 0 and g == 0
                    last = sg == N_SG - 1
                    if g in ACT_GS:
                        emit_act(sg, g, 2 if (first or last) else 1)
                    else:
                        emit_dve(sg, g, 4 if first else (2 if last else 1))

    nc.compile()
    return nc


def build_nofact():
    """Fallback for a non-factorizable rule base: one-hot matmul + exp
    for all 16 groups (the previously validated path)."""
    nc = bacc.Bacc("TRN2", target_bir_lowering=False, debug=False, num_devices=N_CORES)

    oh_ext = nc.dram_tensor("onehot", [K, RPC], BF16, kind="ExternalInput")
    xcs_ext = nc.dram_tensor("xcs", [K, XCS_W], F32, kind="ExternalInput")
    out_ext = nc.dram_tensor("out", [NUM_SAM, RPC], OUT_DT, kind="ExternalOutput")

    with tile.TileContext(nc) as tc:
        with (
            tc.tile_pool(name="const", bufs=1) as cpool,
            tc.tile_pool(name="stage", bufs=4) as spool,
            tc.tile_pool(name="psum", bufs=2, space="PSUM") as ppool,
        ):
            xcs = cpool.tile([K, XCS_W], F32)
            nc.sync.dma_start(out=xcs[:], in_=xcs_ext[:])

            oh = cpool.tile([K, RPC], BF16)
            chunks = [(0, MM_N), (MM_N, 2560), (2688, 2560), (5248, 2944)]
            for c0, csz in chunks:
                nc.scalar.dma_start(
                    out=oh[:, c0 : c0 + csz], in_=oh_ext[:, c0 : c0 + csz]
                )

            lhs = _prep_lhs(nc, cpool, xcs, K)

            for sg in range(N_SG):
                lhsT = lhs[:, sg * 128 : (sg + 1) * 128]
                for g in range(N_GRP):
                    stg = spool.tile([128, EXP_N], OUT_DT)
                    out_slice = out_ext[
                        sg * 128 : (sg + 1) * 128, g * EXP_N : (g + 1) * EXP_N
                    ]
                    ps = ppool.tile([128, EXP_N], F32, tag="ps")
                    for j in range(N_MM):
                        rt = g * N_MM + j
                        nc.tensor.matmul(
                            ps[:, j * MM_N : (j + 1) * MM_N],
                            lhsT,
                            oh[:, rt * MM_N : (rt + 1) * MM_N],
                            start=True, stop=True,
                        )
                    nc.scalar.activation(stg[:], ps[:], Exp)
                    if sg == N_SG - 1 and g == N_GRP - 1:
                        h = EXP_N // 2
                        nc.sync.dma_start(out=out_slice[:, :h], in_=stg[:, :h])
                        nc.sync.dma_start(out=out_slice[:, h:], in_=stg[:, h:])
                    else:
                        nc.sync.dma_start(out=out_slice, in_=stg[:])

    nc.compile()
    return nc


def _is_factorizable(fs):
    """fs[r, 0:4] depends only on r>>8 and fs[r, 4:8] only on r&255."""
    a = fs[:, :D_A].reshape(N_HI, N_LO, D_A)
    b = fs[:, D_A:].reshape(N_HI, N_LO, D_A)
    return bool((a == a[:, :1]).all() and (b == b[:1]).all())


def _prep_in_maps(model_input, center, spread, fs_ind):
    model_input = np.ascontiguousarray(model_input, dtype=np.float32)
    center = np.ascontiguousarray(center, dtype=np.float32)
    spread = np.ascontiguousarray(spread, dtype=np.float32)
    fs = np.clip(np.asarray(fs_ind), 0, NUM_FS - 1).astype(np.int64)

    # xcs row k = d*4+f holds x[s, d] (cols 0:512), center[f, d], spread[f, d]
    xcs = np.empty((K, XCS_W), dtype=np.float32)
    xcs[:, :NUM_SAM] = np.repeat(model_input.T, NUM_FS, axis=0)
    xcs[:, NUM_SAM] = center.T.reshape(K)
    xcs[:, NUM_SAM + 1] = spread.T.reshape(K)

    fact = _is_factorizable(fs)
    r = np.arange(NUM_RULE)
    # full one-hot rule encoding, entries -1 (carry the exponent's sign)
    oh = np.zeros((K, NUM_RULE), dtype=ml_dtypes.bfloat16)
    for d in range(IN_DIM):
        oh[d * NUM_FS + fs[:, d], r] = -1.0

    maps = []
    if fact:
        xcsb = np.ascontiguousarray(xcs[K // 2 : K, :])
        oha = np.zeros((K // 2, N_HI), dtype=ml_dtypes.bfloat16)
        ohb = np.zeros((K // 2, N_LO), dtype=ml_dtypes.bfloat16)
        hi_rep = fs[::N_LO, :D_A]
        lo_rep = fs[:N_LO, D_A:]
        for d in range(D_A):
            oha[d * NUM_FS + hi_rep[:, d], np.arange(N_HI)] = -1.0
            ohb[d * NUM_FS + lo_rep[:, d], np.arange(N_LO)] = -1.0
        for i in range(N_CORES):
            ohab = np.concatenate(
                [oha[:, i * HI_PC : (i + 1) * HI_PC], ohb], axis=1
            )
            ohact = np.concatenate(
                [
                    oh[:, i * RPC + g * EXP_N : i * RPC + (g + 1) * EXP_N]
                    for g in ACT_GS
                ],
                axis=1,
            )
            maps.append(
                {
                    "xcs": xcs,
                    "xcsb": xcsb,
                    "ohab": np.ascontiguousarray(ohab),
                    "ohact": np.ascontiguousarray(ohact),
                }
            )
    else:
        for i in range(N_CORES):
            maps.append(
                {
                    "onehot": np.ascontiguousarray(oh[:, i * RPC : (i + 1) * RPC]),
                    "xcs": xcs,
                }
            )
    return fact, maps


def _run(inputs, trace=False, **spmd_kwargs):
    fact, in_maps = _prep_in_maps(
        inputs["model_input"], inputs["center"], inputs["spread"], inputs["fs_ind"]
    )
    nc = build_fact() if fact else build_nofact()
    res = run_bass_kernel_spmd(
        nc, in_maps, core_ids=list(range(N_CORES)), trace=trace, **spmd_kwargs
    )
    out = np.concatenate(
        [res.results[i]["out"].astype(np.float32) for i in range(N_CORES)], axis=1
    )
    return out, res


def kernel(model_input, center, spread, fs_ind):
    out, _ = _run(
        {
            "model_input": model_input,
            "center": center,
            "spread": spread,
            "fs_ind": fs_ind,
        }
    )
    return out


# revision 11
# speedup vs baseline: 1.1139x; 1.0418x over previous
"""Trainium2 Bass kernel for the Antecedent (fuzzy firing strength) problem.

fir[s, r] = exp(sum_d logmv[s, fs_ind[r, d], d])
with logmv[s, f, d] = -(x[s,d] - c[f,d])^2 / (2 * spread[f,d]^2)

For the FuCo-FRB cartesian rule base, fs_ind factorizes: fs_ind[r, 0:4]
depends only on hi = r>>8 and fs_ind[r, 4:8] only on lo = r&255, so
    fir[s, r] = A[s, hi] * B[s, lo]
with A, B tiny per-sample tables (per core: 32 hi codes, 256 lo codes)
computed via one-hot matmuls + exp.

Sharding: rules split across the 8 cores (8192 rules each); samples
replicated. Production of the 16 [128, 2048] output groups per core is
spread over four engines so the output stores stream continuously:
  - 8 groups: VectorE broadcast multiply A[s,hi]*B[s,lo] (one
    TENSOR_TENSOR per group), stored via the Sync HWDGE queue;
  - 6 groups: TensorE one-hot matmul (K=32) + ScalarE Exp, stored via
    the Scalar HWDGE queue right after each exp in-stream;
  - 2 groups: the same broadcast multiply on the GpSimd (Pool) engine,
    stored via its SWDGE queue (slow engine, but these run early and
    off the critical path);
  - the single xcs input DMA carries x/center and a host-precomputed
    1/(spread*sqrt(2)) column, plus a duplicate of the factor-B rows at
    base partition 32 so the K=16 B matmul needs no extra transfer;
  - first/last groups use split ops + split DMAs to pull the first
    store earlier and shrink the drain tail.
Output is bf16 (fir in (0,1]; rel err ~2e-3), upcast to f32 on the host.
"""

import sys

if "/opt/trn_rl_repo" not in sys.path:
    sys.path.insert(0, "/opt/trn_rl_repo")

import ml_dtypes
import numpy as np

import concourse.bacc as bacc
import concourse.mybir as mybir
import concourse.tile as tile
from concourse.bass_utils import run_bass_kernel_spmd

NUM_SAM = 512
IN_DIM = 8
NUM_FS = 4
NUM_RULE = 65536
K = NUM_FS * IN_DIM  # 32 contraction size
N_CORES = 8
RPC = NUM_RULE // N_CORES  # 8192 rules per core

F32 = mybir.dt.float32
BF16 = mybir.dt.bfloat16
OUT_DT = BF16

N_SG = NUM_SAM // 128  # 4 sample groups of 128 (partition dim)
N_MM = 4               # matmuls per exp group (512 rules each)
MM_N = 512
EXP_N = N_MM * MM_N    # 2048 rules per group
N_GRP = RPC // EXP_N   # 4 rule groups per sample group

D_A = IN_DIM // 2      # factor A: dims 0..3 (k rows 0..15)
N_HI = NUM_FS**D_A     # 256 A-codes globally
N_LO = NUM_FS**D_A     # 256 B-codes
HI_PC = RPC // N_LO    # 32 hi codes per core

XCS_W = NUM_SAM + 2    # x cols + center col + 1/(s*sqrt2) col
KX = K + K // 2        # 32 rows + 16 duplicated factor-B rows (bp32)

ACT_GS = (2, 3)        # rule-groups with one-hot input (matmul+exp path)
# (sg, g) -> producing engine path
PATH = {}
for _sg in range(4):
    for _g in range(4):
        PATH[(_sg, _g)] = "dve" if _g < 2 else "act"
PATH[(3, 2)] = "dve"   # sg3 runs entirely off the A/B tables
PATH[(3, 3)] = "pool"
PATH[(3, 1)] = "pool"

RSQRT2 = 0.7071067811865476
Exp = mybir.ActivationFunctionType.Exp
Mult = mybir.AluOpType.mult


def build_fact():
    nc = bacc.Bacc("TRN2", target_bir_lowering=False, debug=False, num_devices=N_CORES)

    xcs_ext = nc.dram_tensor("xcs", [KX, XCS_W], F32, kind="ExternalInput")
    ohab_ext = nc.dram_tensor("ohab", [K, HI_PC + N_LO], BF16, kind="ExternalInput")
    # one-hot for the ACT-path rule groups, packed in ACT_GS order
    ohact_ext = nc.dram_tensor("ohact", [K, len(ACT_GS) * EXP_N], BF16, kind="ExternalInput")
    out_ext = nc.dram_tensor("out", [NUM_SAM, RPC], OUT_DT, kind="ExternalOutput")

    with tile.TileContext(nc) as tc:
        with (
            tc.tile_pool(name="const", bufs=1) as cpool,
            tc.tile_pool(name="stgv", bufs=3) as svp,
            tc.tile_pool(name="stga", bufs=3) as sap,
            tc.tile_pool(name="stgp", bufs=2) as spp,
            tc.tile_pool(name="psum", bufs=2, space="PSUM") as ppool,
        ):
            # ---- input DMAs, one per issue queue ----
            xcs = cpool.tile([K, XCS_W], F32)
            nc.sync.dma_start(out=xcs[:], in_=xcs_ext[0:K, :])
            ohab = cpool.tile([K, HI_PC + N_LO], BF16)
            nc.scalar.dma_start(out=ohab[:], in_=ohab_ext[:])
            ohact = cpool.tile([K, len(ACT_GS) * EXP_N], BF16)
            for ci in range(len(ACT_GS)):
                nc.gpsimd.dma_start(
                    out=ohact[:, ci * EXP_N : (ci + 1) * EXP_N],
                    in_=ohact_ext[:, ci * EXP_N : (ci + 1) * EXP_N],
                )

            # ---- d2[k, s] = ((x - c) / (s*sqrt2))^2 as bf16 [K, 512] ----
            d2 = cpool.tile([K, NUM_SAM], F32)
            lhs = cpool.tile([K, NUM_SAM], BF16)
            nc.vector.tensor_scalar(
                d2[:], xcs[:, 0:NUM_SAM],
                xcs[:, NUM_SAM : NUM_SAM + 1], xcs[:, NUM_SAM + 1 : NUM_SAM + 2],
                mybir.AluOpType.subtract, Mult,
            )
            nc.vector.tensor_mul(lhs[:], d2[:], d2[:])

            # ---- A/B tables, per sg: two tiny matmuls then one exp ----
            ps_ab = ppool.tile([128, EXP_N], F32, tag="ps")
            ab_tiles = []
            for sg in range(N_SG):
                s0 = sg * MM_N
                sl = slice(sg * 128, (sg + 1) * 128)
                nc.tensor.matmul(
                    ps_ab[:, s0 : s0 + HI_PC + N_LO],
                    lhs[:, sl], ohab[:],
                    start=True, stop=True,
                )
                ab = cpool.tile([128, HI_PC + N_LO], BF16, name=f"ab{sg}")
                nc.scalar.activation(ab[:], ps_ab[:, s0 : s0 + HI_PC + N_LO], Exp)
                ab_tiles.append(ab)

            # ---- main loop ----
            def bcast_tt(eng, stg, ab, g, h0, nh):
                """stg[:, h0*256:(h0+nh)*256] = A[:, g*8+h] * B via one TT."""
                Ab = (
                    ab[:, g * 8 + h0 : g * 8 + h0 + nh]
                    .rearrange("p (h o) -> p h o", o=1)
                    .broadcast_to([128, nh, N_LO])
                )
                Bb = (
                    ab[:, HI_PC : HI_PC + N_LO]
                    .rearrange("p (o n) -> p o n", o=1)
                    .broadcast_to([128, nh, N_LO])
                )
                o3 = stg[:, h0 * N_LO : (h0 + nh) * N_LO].rearrange(
                    "p (h n) -> p h n", h=nh
                )
                eng.tensor_tensor(o3, Bb, Ab, Mult)

            def orow_of(sg, g):
                return out_ext[
                    sg * 128 : (sg + 1) * 128, g * EXP_N : (g + 1) * EXP_N
                ]

            def emit_dve(sg, g, nsplit=1):
                stg = svp.tile([128, EXP_N], OUT_DT, name="svstg")
                orow = orow_of(sg, g)
                hs = 8 // nsplit
                for p in range(nsplit):
                    bcast_tt(nc.vector, stg, ab_tiles[sg], g, p * hs, hs)
                    nc.sync.dma_start(
                        out=orow[:, p * hs * N_LO : (p + 1) * hs * N_LO],
                        in_=stg[:, p * hs * N_LO : (p + 1) * hs * N_LO],
                    )

            def emit_pool(sg, g):
                stg = spp.tile([128, EXP_N], OUT_DT, name="spstg")
                orow = orow_of(sg, g)
                for p in range(2):
                    bcast_tt(nc.gpsimd, stg, ab_tiles[sg], g, p * 4, 4)
                    nc.gpsimd.dma_start(
                        out=orow[:, p * 4 * N_LO : (p + 1) * 4 * N_LO],
                        in_=stg[:, p * 4 * N_LO : (p + 1) * 4 * N_LO],
                    )

            def emit_act(sg, g, nsplit=1):
                ci = ACT_GS.index(g)
                lhsT = lhs[0:K, sg * 128 : (sg + 1) * 128]
                ps = ppool.tile([128, EXP_N], F32, tag="ps", name="ps")
                for j in range(N_MM):
                    nc.tensor.matmul(
                        ps[:, j * MM_N : (j + 1) * MM_N],
                        lhsT,
                        ohact[:, ci * EXP_N + j * MM_N : ci * EXP_N + (j + 1) * MM_N],
                        start=True, stop=True,
                    )
                stg = sap.tile([128, EXP_N], OUT_DT, name="sastg")
                orow = orow_of(sg, g)
                w = EXP_N // nsplit
                for p in range(nsplit):
                    nc.scalar.activation(
                        stg[:, p * w : (p + 1) * w], ps[:, p * w : (p + 1) * w], Exp
                    )
                    nc.scalar.dma_start(
                        out=orow[:, p * w : (p + 1) * w],
                        in_=stg[:, p * w : (p + 1) * w],
                    )

            order = [
                (0, 0), (0, 2), (0, 1), (0, 3),
                (3, 1), (1, 0), (1, 2), (1, 1),
                (1, 3), (3, 3), (2, 0), (2, 2),
                (2, 1), (2, 3), (3, 0), (3, 2),
            ]
            last_of = {}
            for sg, g in order:
                last_of[PATH[(sg, g)]] = (sg, g)
            for sg, g in order:
                path = PATH[(sg, g)]
                first = (sg, g) == order[0]
                last = last_of[path] == (sg, g)
                if path == "dve":
                    emit_dve(sg, g, 4 if first else (2 if last else 1))
                elif path == "act":
                    emit_act(sg, g, 2 if last else 1)
                else:
                    emit_pool(sg, g)

    nc.compile()
    return nc


def build_nofact():
    """Fallback for a non-factorizable rule base: one-hot matmul + exp
    for all 16 groups (the previously validated path)."""
    nc = bacc.Bacc("TRN2", target_bir_lowering=False, debug=False, num_devices=N_CORES)

    oh_ext = nc.dram_tensor("onehot", [K, RPC], BF16, kind="ExternalInput")
    xcs_ext = nc.dram_tensor("xcs", [KX, XCS_W], F32, kind="ExternalInput")
    out_ext = nc.dram_tensor("out", [NUM_SAM, RPC], OUT_DT, kind="ExternalOutput")

    with tile.TileContext(nc) as tc:
        with (
            tc.tile_pool(name="const", bufs=1) as cpool,
            tc.tile_pool(name="stage", bufs=4) as spool,
            tc.tile_pool(name="psum", bufs=2, space="PSUM") as ppool,
        ):
            xcs = cpool.tile([KX, XCS_W], F32)
            nc.sync.dma_start(out=xcs[:], in_=xcs_ext[:])

            oh = cpool.tile([K, RPC], BF16)
            chunks = [(0, MM_N), (MM_N, 2560), (2688, 2560), (5248, 2944)]
            for c0, csz in chunks:
                nc.scalar.dma_start(
                    out=oh[:, c0 : c0 + csz], in_=oh_ext[:, c0 : c0 + csz]
                )

            d2 = cpool.tile([KX, NUM_SAM], F32)
            lhsx = cpool.tile([KX, NUM_SAM], BF16)
            nc.vector.tensor_scalar(
                d2[:], xcs[:, 0:NUM_SAM],
                xcs[:, NUM_SAM : NUM_SAM + 1], xcs[:, NUM_SAM + 1 : NUM_SAM + 2],
                mybir.AluOpType.subtract, Mult,
            )
            nc.vector.tensor_mul(lhsx[:], d2[:], d2[:])

            for sg in range(N_SG):
                lhsT = lhsx[0:K, sg * 128 : (sg + 1) * 128]
                for g in range(N_GRP):
                    stg = spool.tile([128, EXP_N], OUT_DT)
                    out_slice = out_ext[
                        sg * 128 : (sg + 1) * 128, g * EXP_N : (g + 1) * EXP_N
                    ]
                    ps = ppool.tile([128, EXP_N], F32, tag="ps")
                    for j in range(N_MM):
                        rt = g * N_MM + j
                        nc.tensor.matmul(
                            ps[:, j * MM_N : (j + 1) * MM_N],
                            lhsT,
                            oh[:, rt * MM_N : (rt + 1) * MM_N],
                            start=True, stop=True,
                        )
                    nc.scalar.activation(stg[:], ps[:], Exp)
                    if sg == N_SG - 1 and g == N_GRP - 1:
                        h = EXP_N // 2
                        nc.sync.dma_start(out=out_slice[:, :h], in_=stg[:, :h])
                        nc.sync.dma_start(out=out_slice[:, h:], in_=stg[:, h:])
                    else:
                        nc.sync.dma_start(out=out_slice, in_=stg[:])

    nc.compile()
    return nc


def _is_factorizable(fs):
    """fs[r, 0:4] depends only on r>>8 and fs[r, 4:8] only on r&255."""
    a = fs[:, :D_A].reshape(N_HI, N_LO, D_A)
    b = fs[:, D_A:].reshape(N_HI, N_LO, D_A)
    return bool((a == a[:, :1]).all() and (b == b[:1]).all())


def _prep_in_maps(model_input, center, spread, fs_ind):
    model_input = np.ascontiguousarray(model_input, dtype=np.float32)
    center = np.ascontiguousarray(center, dtype=np.float32)
    spread = np.ascontiguousarray(spread, dtype=np.float32)
    fs = np.clip(np.asarray(fs_ind), 0, NUM_FS - 1).astype(np.int64)

    # xcs row k = d*4+f: x[s, d] (cols 0:512), center[f, d], 1/(s*sqrt2);
    # rows 32:48 duplicate rows 16:32 (factor-B dims at base partition 32)
    xcs = np.empty((KX, XCS_W), dtype=np.float32)
    xcs[:K, :NUM_SAM] = np.repeat(model_input.T, NUM_FS, axis=0)
    xcs[:K, NUM_SAM] = center.T.reshape(K)
    xcs[:K, NUM_SAM + 1] = RSQRT2 / spread.T.reshape(K)
    xcs[K:KX] = xcs[K // 2 : K]

    fact = _is_factorizable(fs)
    r = np.arange(NUM_RULE)
    # full one-hot rule encoding, entries -1 (carry the exponent's sign)
    oh = np.zeros((K, NUM_RULE), dtype=ml_dtypes.bfloat16)
    for d in range(IN_DIM):
        oh[d * NUM_FS + fs[:, d], r] = -1.0

    maps = []
    if fact:
        oha = np.zeros((K // 2, N_HI), dtype=ml_dtypes.bfloat16)
        ohb = np.zeros((K // 2, N_LO), dtype=ml_dtypes.bfloat16)
        hi_rep = fs[::N_LO, :D_A]
        lo_rep = fs[:N_LO, D_A:]
        for d in range(D_A):
            oha[d * NUM_FS + hi_rep[:, d], np.arange(N_HI)] = -1.0
            ohb[d * NUM_FS + lo_rep[:, d], np.arange(N_LO)] = -1.0
        for i in range(N_CORES):
            ohab = np.zeros((K, HI_PC + N_LO), dtype=ml_dtypes.bfloat16)
            ohab[: K // 2, :HI_PC] = oha[:, i * HI_PC : (i + 1) * HI_PC]
            ohab[K // 2 :, HI_PC:] = ohb
            ohact = np.concatenate(
                [
                    oh[:, i * RPC + g * EXP_N : i * RPC + (g + 1) * EXP_N]
                    for g in ACT_GS
                ],
                axis=1,
            )
            maps.append(
                {
                    "xcs": xcs,
                    "ohab": np.ascontiguousarray(ohab),
                    "ohact": np.ascontiguousarray(ohact),
                }
            )
    else:
        for i in range(N_CORES):
            maps.append(
                {
                    "onehot": np.ascontiguousarray(oh[:, i * RPC : (i + 1) * RPC]),
                    "xcs": xcs,
                }
            )
    return fact, maps


def _run(inputs, trace=False, **spmd_kwargs):
    fact, in_maps = _prep_in_maps(
        inputs["model_input"], inputs["center"], inputs["spread"], inputs["fs_ind"]
    )
    nc = build_fact() if fact else build_nofact()
    res = run_bass_kernel_spmd(
        nc, in_maps, core_ids=list(range(N_CORES)), trace=trace, **spmd_kwargs
    )
    out = np.concatenate(
        [res.results[i]["out"].astype(np.float32) for i in range(N_CORES)], axis=1
    )
    return out, res


def kernel(model_input, center, spread, fs_ind):
    out, _ = _run(
        {
            "model_input": model_input,
            "center": center,
            "spread": spread,
            "fs_ind": fs_ind,
        }
    )
    return out
